# revision 15
# baseline (speedup 1.0000x reference)
"""AGCN (adaptive graph conv) distributed Bass kernel for 8 TRN2 NeuronCores.

Sharding: data-parallel over batch B=32 -> 4 batches/core, no collectives.

Host precomputes the adjacency S = softmax(relu(nv1@nv2)) AND S^2, so both
graph hops become x-stationary matmuls straight from the DMA streams:
  Y1^T[(b,i), n] = sum_m x[m,(b,i)]^T  S^T[m, n]
  U2^T[(b,i), n] = sum_m x[m,(b,i)]^T (S^2)^T[m, n]
This removes every PE transpose and the Y1 round-trip of the v1 kernel.

The hop lhsT column layout is rotated (xwx has 320 cols = [b0 b1 b2 b3 b0])
so the Y-slabs pair batches (0,1),(2,3) while the U-slabs pair (1,2),(3,0).
All PSUM->SBUF drains then land partition-aligned in per-batch combine tiles
xgtYU[b] = even b: [Y_b; U_b], odd b: [U_b; Y_b] (rhs blocks swapped to
match); paired accumulators drain in single strided ops.

Chebyshev fold (host): out = x(W0-W2) + Y1 W1 + U2 (2 W2) + bias.

Combine per (nt, b): Z[n,(o,d)] = YU-pair matmul (K=128) + x^T matmul (K=64).
Drains: zs PSUM->SBUF copies run exclusively on ACT (a pure FIFO, so the
2-deep pZ ring never waits behind tree work), then the emb-weighted d-reduce
runs pair-batched on DVE/Pool.

Pipeline: n is processed in 5 column chunks [256,512,512,512,208]; chunk c's
combine units interleave with chunk c+1's hop matmuls on the PE, the small
first chunk gets combine started early and the small last chunk shrinks the
drain-only tail. Warmup matmuls fill every DMA-paced stretch of the first
chunk so the PE p-state never drops.
"""

import os
import sys

for _p in ("/opt/trn_rl_repo",):
    if _p not in sys.path:
        sys.path.insert(0, _p)

from contextlib import ExitStack

import ml_dtypes
import numpy as np

import concourse.bass as bass  # noqa: F401  (bass import keeps mybir registry happy)
import concourse.tile as tile
from concourse import bacc, mybir
from concourse.bass_utils import run_bass_kernel_spmd

BF16 = ml_dtypes.bfloat16

B, N, DIN, DOUT, EMB, CHEB = 32, 2000, 64, 64, 16, 3
CORES = 8
BLOC = B // CORES          # 4 batches per core
P = 128
NT = (N + P - 1) // P      # 16 node tiles (last = 80 rows)
DO = EMB * DOUT            # 1024 (o,d) free, d innermost
NPAD = NT * P              # 2048 (padded rows for the m/contraction streams)
CW = [256, 512, 512, 512, 128, 80]    # chunk widths (cols of n)
COF = [0, 256, 768, 1280, 1792, 1920]  # chunk col offsets
CT0 = [0, 2, 6, 10, 14, 15]           # first tile of each chunk
CNT = [2, 4, 4, 4, 1, 1]              # tiles per chunk
NCH = 6
WARMUP = int(os.environ.get("WARMUP", "40"))
SPRINKLE = int(os.environ.get("SPRINKLE", "10"))

# tree engine per unit PAIR (30 non-tail pairs, chunk-major): g = DVE, h = Pool
PAIRS = os.environ.get(
    "PAIRS",
    "gghg" + "gghggghg" * 3 + "gh",
)
# tail unit drain paths (4 units of the last tile):
#   a: zs=ACT ze/tree=DVE   b: zs=ACT ze=DVE tree=Pool
#   f: fused DVE mult, tree=DVE   p: fused DVE mult, tree=Pool
TAILP = os.environ.get("TAILP", "apbf")


def _tsz(t: int) -> int:
    return min(P, N - t * P)


def _build():
    nc = bacc.Bacc("TRN2", target_bir_lowering=False, debug=False)
    f32, bf16 = mybir.dt.float32, mybir.dt.bfloat16
    AF = mybir.ActivationFunctionType
    OP = mybir.AluOpType

    xwx = nc.declare_dram_parameter("xwx", [NPAD, 320], bf16, isOutput=False)
    std = nc.declare_dram_parameter("std", [NPAD, N], bf16, isOutput=False)
    s2d = nc.declare_dram_parameter("s2d", [NPAD, N], bf16, isOutput=False)
    xtp = nc.declare_dram_parameter("xtp", [2, P, N], bf16, isOutput=False)
    wf3 = nc.declare_dram_parameter("wf3", [3, P, DO], bf16, isOutput=False)
    embd = nc.declare_dram_parameter("embd", [NPAD, EMB], bf16, isOutput=False)
    biasd = nc.declare_dram_parameter("biasd", [NPAD, DOUT], bf16, isOutput=False)
    outp = nc.declare_dram_parameter("out", [N, BLOC, DOUT], bf16, isOutput=True)

    with tile.TileContext(nc) as tc, ExitStack() as ctx:
        sing = ctx.enter_context(tc.tile_pool(name="sing", bufs=1))
        wrk = ctx.enter_context(tc.tile_pool(name="wrk", bufs=6))
        wrk2 = ctx.enter_context(tc.tile_pool(name="wrk2", bufs=3))
        ps = ctx.enter_context(tc.tile_pool(name="ps", bufs=1, space="PSUM"))

        # persistent SBUF
        sts = sing.tile([P, NT, N], bf16)       # S^T    [m-part, mt, n]
        s2s = sing.tile([P, NT, N], bf16)       # (S^2)^T
        xws = sing.tile([P, NT, 320], bf16)     # x (b,i) cols + 64-col rotation
        xgtYU = sing.tile([P, BLOC, N], bf16)   # per-b [Y;U] / [U;Y] pair slabs
        xgtX = sing.tile([P, 2, N], bf16)       # x^T pair slabs
        wfs = sing.tile([P, 3, DO], bf16)       # [B;C], [C;B], [A;A]
        emb16 = sing.tile([P, NT, EMB], bf16)
        bias16 = sing.tile([P, NT, DOUT], bf16)
        warm = sing.tile([P, P], bf16)          # zeroed warmup fuel

        # absorb one-time engine init costs off the critical path
        nc.vector.memset(warm[:, :], 0.0)
        pre1 = wrk.tile([P, 8], bf16, tag="pre", name="pre1")
        nc.scalar.activation(pre1[:, :], warm[:, 0:8], AF.Copy)  # ACT table load
        pre2 = wrk.tile([P, 8], bf16, tag="pre2", name="pre2")
        nc.gpsimd.memset(pre2[:, :], 0.0)  # Pool Q7 spin-up

        # ---- DMA program ----
        def xw_blk(k):
            nc.sync.dma_start(
                out=xws[:, 4 * k : 4 * k + 4, :],
                in_=xwx[512 * k : 512 * k + 512, :].rearrange("(t p) c -> p t c", p=P),
            )

        def stream_blk(c, k):
            q0, qw = COF[c], CW[c]
            for src, dst in ((std, sts), (s2d, s2s)):
                nc.sync.dma_start(
                    out=dst[:, 4 * k : 4 * k + 4, q0 : q0 + qw],
                    in_=src[512 * k : 512 * k + 512, q0 : q0 + qw].rearrange(
                        "(t p) c -> p t c", p=P
                    ),
                )

        for k in range(4):
            xw_blk(k)
            stream_blk(0, k)
        stream_blk(1, 0)
        nc.sync.dma_start(out=xgtX[:, :, :], in_=xtp[:, :, :].rearrange("j p n -> p j n"))
        stream_blk(1, 1)
        nc.sync.dma_start(out=wfs[:, :, :], in_=wf3[:, :, :].rearrange("c p f -> p c f"))
        stream_blk(1, 2)
        nc.sync.dma_start(
            out=emb16[:, :, :], in_=embd[:, :].rearrange("(t p) e -> p t e", p=P)
        )
        nc.sync.dma_start(
            out=bias16[:, :, :], in_=biasd[:, :].rearrange("(t p) e -> p t e", p=P)
        )
        stream_blk(1, 3)
        for c in range(2, NCH):
            for k in range(4):
                stream_blk(c, k)

        # ---- PE warmup (no deps: reads whatever is in `warm`) ----
        pZw = ps.tile([P, DO], f32, tag="Z0", name="pZw")

        def warm_mms(n):
            for _ in range(n):
                nc.tensor.matmul(
                    pZw[:, 0:P], lhsT=warm[:, :], rhs=warm[:, :],
                    start=True, stop=True,
                )

        warm_mms(WARMUP)

        # ---- hops ----
        # accumulators in one 4-bank tile: 0=YA(b0,b1) 1=YB(b2,b3)
        #                                  2=UA(b1,b2) 3=UB(b3,b0)
        ACC_C0 = [0, 128, 64, 192]

        def hop_mm(pH, c, acc, mt):
            q0, qw = COF[c], CW[c]
            src = sts if acc < 2 else s2s
            nc.tensor.matmul(
                pH[:, acc, :qw],
                lhsT=xws[:, mt, ACC_C0[acc] : ACC_C0[acc] + 128],
                rhs=src[:, mt, q0 : q0 + qw],
                start=(mt == 0),
                stop=(mt == NT - 1),
            )

        def _sap(base, stride, n=2):
            return bass.AP(
                tensor=base.tensor,
                offset=base.offset,
                ap=[base.ap[0], [stride, n], base.ap[1]],
            )

        def pair_drains(pH, c, gi):
            """Drain accumulator pair gi (0: YA+UA -> b0,b1,b2-hi;
            1: YB+UB -> b2-lo,b3,b0-hi). Partition-aligned by construction.
            All on DVE/Pool so the ACT zs FIFO stays unclogged."""
            q0, qw = COF[c], CW[c]
            if gi == 0:
                moves = [
                    (_sap(pH[0:64, 0, 0:qw], 1024),
                     _sap(xgtYU[0:64, 0, q0 : q0 + qw], N), "P"),
                    (_sap(pH[64:P, 0, 0:qw], 1024),
                     _sap(xgtYU[64:P, 1, q0 : q0 + qw], N), "D"),
                ]
            else:
                moves = [
                    (_sap(pH[0:64, 1, 0:qw], 1024),
                     _sap(xgtYU[0:64, 2, q0 : q0 + qw], N), "D"),
                    (pH[64:P, 1, 0:qw], xgtYU[64:P, 3, q0 : q0 + qw], "D"),
                    (pH[64:P, 3, 0:qw], xgtYU[64:P, 0, q0 : q0 + qw], "D"),
                ]
            for src, dst, eng in moves:
                if eng == "P":
                    nc.gpsimd.tensor_copy(dst, src)
                else:
                    nc.vector.tensor_copy(dst, src)

        # ---- combine ----
        obs = {}

        def ob_for(nt):
            if nt not in obs:
                obs[nt] = wrk2.tile([P, BLOC, DOUT], bf16, tag="ob", name="ob")
            return obs[nt]

        def finish_tile(nt):
            pn = _tsz(nt)
            bsl = bias16[:pn, nt, :]
            bB = bass.AP(
                tensor=bsl.tensor,
                offset=bsl.offset,
                ap=[bsl.ap[0], [0, BLOC], bsl.ap[1]],
            )
            ob = obs.pop(nt)
            nc.vector.tensor_tensor(ob[:pn], ob[:pn], bB, OP.add)
            nc.sync.dma_start(out=outp[nt * P : nt * P + pn, :, :], in_=ob[:pn, :, :])

        tailps = []

        def unit_mms(nt, b, halves):
            pn = _tsz(nt)
            nsl = slice(nt * P, nt * P + pn)
            p0 = (b % 2) * DIN
            for half in range(2):
                fsl = slice(half * 512, half * 512 + 512)
                nc.tensor.matmul(
                    halves[half],
                    lhsT=xgtYU[:, b, nsl],
                    rhs=wfs[:, b % 2, fsl],
                    start=True,
                    stop=False,
                )
                nc.tensor.matmul(
                    halves[half],
                    lhsT=xgtX[p0 : p0 + DIN, b // 2, nsl],
                    rhs=wfs[p0 : p0 + DIN, 2, fsl],
                    start=False,
                    stop=True,
                )

        def unit_pair(nt, bpair, pidx):
            """Two combine units (nt, b0) (nt, b1); zs on ACT per unit, then
            one pair-batched ze and d-reduce tree on DVE or Pool."""
            pn = _tsz(nt)
            path = PAIRS[pidx]
            zs2 = wrk.tile([P, 2, DO], bf16, tag="zs", name="zs2", bufs=3)
            for j, b in enumerate(bpair):
                pZ = ps.tile([P, DO], f32, tag=f"Z{j}", name="pZ")
                unit_mms(nt, b, [pZ[:pn, 0:512], pZ[:pn, 512:1024]])
                nc.scalar.activation(zs2[:pn, j, :], pZ[:pn, :], AF.Copy)
            esl = emb16[:pn, nt, :]
            eeB2 = bass.AP(
                tensor=esl.tensor,
                offset=esl.offset,
                ap=[esl.ap[0], [0, 2], [0, DOUT], esl.ap[1]],
            )
            ze2 = wrk.tile([P, 2, DOUT, EMB], bf16, tag="ze", name="ze2", bufs=3)
            nc.vector.tensor_tensor(
                ze2[:pn], zs2[:pn].rearrange("p b (o d) -> p b o d", d=EMB),
                eeB2, OP.mult,
            )
            eng = nc.gpsimd if path == "h" else nc.vector
            tg = path
            ob = ob_for(nt)
            t8 = wrk.tile([P, 2, DOUT, 8], bf16, tag=f"t8{tg}", name="t8", bufs=2)
            eng.tensor_tensor(t8[:pn], ze2[:pn, :, :, 0:8], ze2[:pn, :, :, 8:16], OP.add)
            t4 = wrk.tile([P, 2, DOUT, 4], bf16, tag=f"t4{tg}", name="t4", bufs=2)
            eng.tensor_tensor(t4[:pn], t8[:pn, :, :, 0:4], t8[:pn, :, :, 4:8], OP.add)
            t2 = wrk.tile([P, 2, DOUT, 2], bf16, tag=f"t2{tg}", name="t2", bufs=2)
            eng.tensor_tensor(t2[:pn], t4[:pn, :, :, 0:2], t4[:pn, :, :, 2:4], OP.add)
            with nc.allow_low_precision(reason="16-term bf16 reduce"):
                for j, b in enumerate(bpair):
                    eng.tensor_tensor(
                        ob[:pn, b, :].rearrange("p (o v) -> p o v", v=1),
                        t2[:pn, j, :, 0:1],
                        t2[:pn, j, :, 1:2],
                        OP.add,
                    )

        def tail_unit(nt, b, tpath, tctx):
            """Unbatched tail unit: drains spread over ACT/DVE/Pool, 4-deep
            pZ ring via the freed hop banks."""
            pn = _tsz(nt)
            if tctx in (1, 3):
                if not tailps:
                    tailps.append(ps.tile([P, 4, 512], f32, tag="H", name="pHt"))
                hj = tctx - 1
                halves = [tailps[0][:pn, hj, :], tailps[0][:pn, hj + 1, :]]
                pZ = None
            else:
                pZ = ps.tile([P, DO], f32, tag=f"Z{tctx // 2}", name="pZt")
                halves = [pZ[:pn, 0:512], pZ[:pn, 512:1024]]
            unit_mms(nt, b, halves)
            esl = emb16[:pn, nt, :]
            eeB = bass.AP(
                tensor=esl.tensor,
                offset=esl.offset,
                ap=[esl.ap[0], [0, DOUT], esl.ap[1]],
            )
            ze = wrk.tile([P, DOUT, EMB], bf16, tag="ze", name="zet", bufs=3)
            if tpath in ("f", "p") and pZ is not None:
                nc.vector.tensor_tensor(
                    ze[:pn], pZ[:pn, :].rearrange("p (o d) -> p o d", d=EMB),
                    eeB, OP.mult,
                )
            else:
                zs = wrk.tile([P, DO], bf16, tag="zs", name="zst", bufs=3)
                if pZ is None:
                    nc.scalar.activation(zs[:pn, 0:512], halves[0], AF.Copy)
                    nc.vector.tensor_copy(zs[:pn, 512:1024], halves[1])
                else:
                    nc.scalar.activation(zs[:pn, :], pZ[:pn, :], AF.Copy)
                nc.vector.tensor_tensor(
                    ze[:pn], zs[:pn, :].rearrange("p (o d) -> p o d", d=EMB),
                    eeB, OP.mult,
                )
            eng = nc.gpsimd if tpath in ("b", "p") else nc.vector
            tg = "h" if tpath in ("b", "p") else "g"
            ob = ob_for(nt)
            t8 = wrk.tile([P, DOUT, 8], bf16, tag=f"t8{tg}", name="t8t", bufs=2)
            eng.tensor_tensor(t8[:pn], ze[:pn, :, 0:8], ze[:pn, :, 8:16], OP.add)
            t4 = wrk.tile([P, DOUT, 4], bf16, tag=f"t4{tg}", name="t4t", bufs=2)
            eng.tensor_tensor(t4[:pn], t8[:pn, :, 0:4], t8[:pn, :, 4:8], OP.add)
            t2 = wrk.tile([P, DOUT, 2], bf16, tag=f"t2{tg}", name="t2t", bufs=2)
            eng.tensor_tensor(t2[:pn], t4[:pn, :, 0:2], t4[:pn, :, 2:4], OP.add)
            with nc.allow_low_precision(reason="16-term bf16 reduce"):
                eng.tensor_tensor(
                    ob[:pn, b, :].rearrange("p (o v) -> p o v", v=1),
                    t2[:pn, :, 0:1],
                    t2[:pn, :, 1:2],
                    OP.add,
                )

        # ---- pipeline ----
        pending = []   # (nt, bpair, pidx)
        pcount = [0]

        def enqueue_chunk(c):
            for t in range(CNT[c]):
                nt = CT0[c] + t
                for bpair in ((1, 0), (3, 2)):
                    pending.append((nt, bpair, pcount[0]))
                    pcount[0] += 1

        def emit_pair():
            if pending:
                nt, bpair, pidx = pending.pop(0)
                unit_pair(nt, bpair, pidx)
                if bpair[0] == 3:
                    finish_tile(nt)
                return True
            return False

        def hop_chunk(c):
            pH = ps.tile([P, 4, 512], f32, tag="H", name=f"pH{c}")
            for mt in range(12):
                for acc in range(4):
                    hop_mm(pH, c, acc, mt)
                # pair-emission cadence tuned to DMA arrival: chunk-1 pairs
                # wait for the wfs/xtp DMAs, later chunks for stream blocks
                want = (c == 1 and mt >= 6) or (c >= 2 and mt % 2 == 1)
                if want:
                    emit_pair()
                elif c == 0 and mt % 2 == 1:
                    warm_mms(SPRINKLE)
            for gi, accs in enumerate(((0, 2), (1, 3))):
                for acc in accs:
                    for mt in range(12, 16):
                        hop_mm(pH, c, acc, mt)
                pair_drains(pH, c, gi)
                if c >= 1:
                    emit_pair()
                if c >= 2:
                    emit_pair()

        for c in range(NCH):
            hop_chunk(c)
            if c < NCH - 1:
                enqueue_chunk(c)
        while emit_pair():
            pass
        # tail: last tile, 4 unbatched units on a 4-deep ring
        tnt = CT0[NCH - 1]
        for k, b in enumerate((1, 0, 3, 2)):
            tail_unit(tnt, b, TAILP[k], k)
        finish_tile(tnt)

    nc.compile()
    return nc


_NC_CACHE: list = []


def _get_nc():
    if not _NC_CACHE:
        _NC_CACHE.append(_build())
    return _NC_CACHE[0]


def _prep_shared(node_embeddings, nodevec1, nodevec2, weights_pool, bias_pool):
    nv1 = np.asarray(nodevec1, np.float32)
    nv2 = np.asarray(nodevec2, np.float32)
    z = np.maximum(nv1 @ nv2, 0.0)
    e = np.exp(z - z.max(axis=1, keepdims=True))
    s = e / e.sum(axis=1, keepdims=True)
    s2 = s @ s
    std = np.zeros((NPAD, N), np.float32)
    std[:N] = s.T
    s2d = np.zeros((NPAD, N), np.float32)
    s2d[:N] = s2.T

    wp = np.asarray(weights_pool, np.float32)  # [EMB, K, I, O]

    def blk(M):  # [EMB, I, O] -> [I, (O, EMB)] d-minor
        return np.transpose(M, (1, 2, 0)).reshape(DIN, DO)

    A = blk(wp[:, 0] - wp[:, 2])
    Bb = blk(wp[:, 1])
    C = blk(2.0 * wp[:, 2])
    wf3 = np.stack(
        [np.vstack([Bb, C]), np.vstack([C, Bb]), np.vstack([A, A])], axis=0
    )

    emb = np.asarray(node_embeddings, np.float32)
    embp = np.zeros((NPAD, EMB), np.float32)
    embp[:N] = emb
    biasp = np.zeros((NPAD, DOUT), np.float32)
    biasp[:N] = emb @ np.asarray(bias_pool, np.float32)
    return {
        "std": std.astype(BF16),
        "s2d": s2d.astype(BF16),
        "wf3": wf3.astype(BF16),
        "embd": embp.astype(BF16),
        "biasd": biasp.astype(BF16),
    }


def _prep_core(x, core):
    xl = np.asarray(x[core * BLOC : (core + 1) * BLOC], np.float32)  # [4, N, 64]
    xw = np.ascontiguousarray(xl.transpose(1, 0, 2).reshape(N, BLOC * DIN))
    xwx = np.zeros((NPAD, 320), np.float32)
    xwx[:N, 0:256] = xw
    xwx[:N, 256:320] = xw[:, 0:64]
    xtp = np.ascontiguousarray(xl.transpose(0, 2, 1).reshape(2, P, N))
    return {"xwx": xwx.astype(BF16), "xtp": xtp.astype(BF16)}


def run(x, node_embeddings, nodevec1, nodevec2, weights_pool, bias_pool, **spmd_kwargs):
    nc = _get_nc()
    shared = _prep_shared(node_embeddings, nodevec1, nodevec2, weights_pool, bias_pool)
    in_maps = [{**shared, **_prep_core(x, c)} for c in range(CORES)]
    res = run_bass_kernel_spmd(nc, in_maps, core_ids=list(range(CORES)), **spmd_kwargs)
    out = np.concatenate(
        [
            np.asarray(res.results[c]["out"], np.float32).transpose(1, 0, 2)
            for c in range(CORES)
        ],
        axis=0,
    )
    return np.ascontiguousarray(out), res


def kernel(x, node_embeddings, nodevec1, nodevec2, weights_pool, bias_pool):
    out, _ = run(x, node_embeddings, nodevec1, nodevec2, weights_pool, bias_pool)
    return out


# revision 17
# speedup vs baseline: 1.1297x; 1.1297x over previous
"""AGCN (adaptive graph conv) distributed Bass kernel for 8 TRN2 NeuronCores.

Sharding: data-parallel over batch B=32 -> 4 batches/core, no collectives.

Host precomputes the adjacency S = softmax(relu(nv1@nv2)) AND S^2, so both
graph hops become x-stationary matmuls straight from the DMA streams:
  Y1^T[(b,i), n] = sum_m x[m,(b,i)]^T  S^T[m, n]
  U2^T[(b,i), n] = sum_m x[m,(b,i)]^T (S^2)^T[m, n]
This removes every PE transpose and the Y1 round-trip of the v1 kernel.

The hop lhsT column layout is rotated (xwx has 320 cols = [b0 b1 b2 b3 b0])
so the Y-slabs pair batches (0,1),(2,3) while the U-slabs pair (1,2),(3,0).
All PSUM->SBUF drains then land partition-aligned in per-batch combine tiles
xgtYU[b] = even b: [Y_b; U_b], odd b: [U_b; Y_b] (rhs blocks swapped to
match); paired accumulators drain in single strided ops.

Chebyshev fold (host): out = x(W0-W2) + Y1 W1 + U2 (2 W2) + bias.

Combine per (nt, b): Z[n,(o,d)] = YU-pair matmul (K=128) + x^T matmul (K=64).
Drains: zs PSUM->SBUF copies run exclusively on ACT (a pure FIFO), then the
emb-weighted d-reduce runs pair-batched on DVE/Pool.

Pipeline: n is processed in 8 narrow 256-col hop chunks so the four hop
accumulators need only 2 PSUM banks, leaving 6 banks for a 3-deep combine pZ
ring that rides out drain-latency jitter. Stream DMAs are issued in 512-col
regions ordered so hops are never starved; the misc inputs (x^T, weights,
emb/bias) land right before the first combine pair, and a burst of buffered
pairs after chunk 2 absorbs any remaining DMA lag. Warmup matmuls keep the
PE p-state pinned high through every DMA-paced stretch.
"""

import os
import sys

for _p in ("/opt/trn_rl_repo",):
    if _p not in sys.path:
        sys.path.insert(0, _p)

from contextlib import ExitStack

import ml_dtypes
import numpy as np

import concourse.bass as bass  # noqa: F401  (bass import keeps mybir registry happy)
import concourse.tile as tile
from concourse import bacc, mybir
from concourse.bass_utils import run_bass_kernel_spmd

BF16 = ml_dtypes.bfloat16

B, N, DIN, DOUT, EMB, CHEB = 32, 2000, 64, 64, 16, 3
CORES = 8
BLOC = B // CORES          # 4 batches per core
P = 128
NT = (N + P - 1) // P      # 16 node tiles (last = 80 rows)
DO = EMB * DOUT            # 1024 (o,d) free, d innermost
NPAD = NT * P              # 2048 (padded rows for the m/contraction streams)
NCH = 8
CW = [256] * 7 + [208]                 # hop chunk widths (cols of n)
COF = [256 * c for c in range(8)]      # chunk col offsets
CT0 = [2 * c for c in range(8)]        # first tile of each chunk
CNT = [2] * 8                          # tiles per chunk
WARMUP = int(os.environ.get("WARMUP", "40"))
SPRINKLE = int(os.environ.get("SPRINKLE", "6"))

# tree engine per unit PAIR (30 non-tail pairs, emission order): g=DVE h=Pool
PAIRS = os.environ.get("PAIRS", "gghggghgghgghggghggghgghgghggh")
# tail unit drain paths (4 units of the last tile):
#   a: zs=ACT ze/tree=DVE   b: zs=ACT ze=DVE tree=Pool
#   f: fused DVE mult, tree=DVE   p: fused DVE mult, tree=Pool
TAILP = os.environ.get("TAILP", "apbf")


def _tsz(t: int) -> int:
    return min(P, N - t * P)


def _build():
    nc = bacc.Bacc("TRN2", target_bir_lowering=False, debug=False)
    f32, bf16 = mybir.dt.float32, mybir.dt.bfloat16
    AF = mybir.ActivationFunctionType
    OP = mybir.AluOpType

    xwx = nc.declare_dram_parameter("xwx", [NPAD, 320], bf16, isOutput=False)
    std = nc.declare_dram_parameter("std", [NPAD, N], bf16, isOutput=False)
    s2d = nc.declare_dram_parameter("s2d", [NPAD, N], bf16, isOutput=False)
    xtp = nc.declare_dram_parameter("xtp", [2, P, N], bf16, isOutput=False)
    wf3 = nc.declare_dram_parameter("wf3", [3, P, DO], bf16, isOutput=False)
    ebd = nc.declare_dram_parameter("ebd", [NPAD, EMB + DOUT], bf16, isOutput=False)
    outp = nc.declare_dram_parameter("out", [N, BLOC, DOUT], bf16, isOutput=True)

    with tile.TileContext(nc) as tc, ExitStack() as ctx:
        sing = ctx.enter_context(tc.tile_pool(name="sing", bufs=1))
        wrk = ctx.enter_context(tc.tile_pool(name="wrk", bufs=6))
        wrk2 = ctx.enter_context(tc.tile_pool(name="wrk2", bufs=3))
        ps = ctx.enter_context(tc.tile_pool(name="ps", bufs=1, space="PSUM"))

        # persistent SBUF
        sts = sing.tile([P, NT, N], bf16)       # S^T    [m-part, mt, n]
        s2s = sing.tile([P, NT, N], bf16)       # (S^2)^T
        xws = sing.tile([P, NT, 320], bf16)     # x (b,i) cols + 64-col rotation
        xgtYU = sing.tile([P, BLOC, N], bf16)   # per-b [Y;U] / [U;Y] pair slabs
        xgtX = sing.tile([P, 2, N], bf16)       # x^T pair slabs
        wfs = sing.tile([P, 3, DO], bf16)       # [B;C], [C;B], [A;A]
        ebs = sing.tile([P, NT, EMB + DOUT], bf16)  # emb | bias per tile
        warm = sing.tile([P, P], bf16)          # zeroed warmup fuel

        # absorb one-time engine init costs off the critical path
        nc.vector.memset(warm[:, :], 0.0)
        pre1 = wrk.tile([P, 8], bf16, tag="pre", name="pre1")
        nc.scalar.activation(pre1[:, :], warm[:, 0:8], AF.Copy)  # ACT table load
        pre2 = wrk.tile([P, 8], bf16, tag="pre2", name="pre2")
        nc.gpsimd.memset(pre2[:, :], 0.0)  # Pool Q7 spin-up

        # ---- DMA program ----
        def stream_blk(cols, k, w):
            for src, dst in ((std, sts), (s2d, s2s)):
                nc.sync.dma_start(
                    out=dst[:, 4 * k : 4 * k + 4, cols : cols + w],
                    in_=src[512 * k : 512 * k + 512, cols : cols + w].rearrange(
                        "(t p) c -> p t c", p=P
                    ),
                )

        nc.sync.dma_start(
            out=xws[:, :, :],
            in_=xwx[:, :].rearrange("(t p) c -> p t c", p=P),
        )
        for k in range(4):
            stream_blk(0, k, 256)          # chunk 0 fuel, fine-grained
        for k in range(4):
            stream_blk(256, k, 512)        # region 1: chunks 1-2
        nc.sync.dma_start(out=xgtX[:, :, :], in_=xtp[:, :, :].rearrange("j p n -> p j n"))
        nc.sync.dma_start(out=wfs[:, :, :], in_=wf3[:, :, :].rearrange("c p f -> p c f"))
        nc.sync.dma_start(
            out=ebs[:, :, :], in_=ebd[:, :].rearrange("(t p) e -> p t e", p=P)
        )
        for cols, w in ((768, 512), (1280, 512), (1792, 208)):
            for k in range(4):
                stream_blk(cols, k, w)

        # ---- PE warmup (fills DMA-paced stretches; p-state stays pinned) ----
        pZw = ps.tile([P, DO], f32, tag="Z0", name="pZw")

        def warm_mms(n):
            for _ in range(n):
                nc.tensor.matmul(
                    pZw[:, 0:P], lhsT=warm[:, :], rhs=warm[:, :],
                    start=True, stop=True,
                )

        warm_mms(WARMUP)

        # ---- hops ----
        # accumulators in one 2-bank tile: 0=YA(b0,b1) 1=YB(b2,b3)
        #                                  2=UA(b1,b2) 3=UB(b3,b0)
        ACC_C0 = [0, 128, 64, 192]

        def hop_mm(pH, c, acc, mt):
            q0, qw = COF[c], CW[c]
            src = sts if acc < 2 else s2s
            nc.tensor.matmul(
                pH[:, acc, :qw],
                lhsT=xws[:, mt, ACC_C0[acc] : ACC_C0[acc] + 128],
                rhs=src[:, mt, q0 : q0 + qw],
                start=(mt == 0),
                stop=(mt == NT - 1),
            )

        def _sap(base, stride, n=2):
            return bass.AP(
                tensor=base.tensor,
                offset=base.offset,
                ap=[base.ap[0], [stride, n], base.ap[1]],
            )

        def pair_drains(pH, c, gi):
            """Drain accumulator pair gi (0: YA+UA, 1: YB+UB) into the
            per-batch combine tiles; partition-aligned by construction.
            On DVE/Pool so the ACT zs FIFO stays unclogged."""
            q0, qw = COF[c], CW[c]
            if gi == 0:
                moves = [
                    (_sap(pH[0:64, 0, 0:qw], 2 * 256),
                     _sap(xgtYU[0:64, 0, q0 : q0 + qw], N), "P"),
                    (_sap(pH[64:P, 0, 0:qw], 2 * 256),
                     _sap(xgtYU[64:P, 1, q0 : q0 + qw], N), "D"),
                ]
            else:
                moves = [
                    (_sap(pH[0:64, 1, 0:qw], 2 * 256),
                     _sap(xgtYU[0:64, 2, q0 : q0 + qw], N), "D"),
                    (pH[64:P, 1, 0:qw], xgtYU[64:P, 3, q0 : q0 + qw], "D"),
                    (pH[64:P, 3, 0:qw], xgtYU[64:P, 0, q0 : q0 + qw], "P"),
                ]
            for src, dst, eng in moves:
                if eng == "P":
                    nc.gpsimd.tensor_copy(dst, src)
                else:
                    nc.vector.tensor_copy(dst, src)

        # ---- combine ----
        obs = {}

        def ob_for(nt):
            if nt not in obs:
                obs[nt] = wrk2.tile([P, BLOC, DOUT], bf16, tag="ob", name="ob")
            return obs[nt]

        def finish_tile(nt):
            pn = _tsz(nt)
            bsl = ebs[:pn, nt, EMB:]
            bB = bass.AP(
                tensor=bsl.tensor,
                offset=bsl.offset,
                ap=[bsl.ap[0], [0, BLOC], bsl.ap[1]],
            )
            ob = obs.pop(nt)
            nc.vector.tensor_tensor(ob[:pn], ob[:pn], bB, OP.add)
            nc.sync.dma_start(out=outp[nt * P : nt * P + pn, :, :], in_=ob[:pn, :, :])

        zring = [0]

        def unit_mms(nt, b, halves):
            pn = _tsz(nt)
            nsl = slice(nt * P, nt * P + pn)
            p0 = (b % 2) * DIN
            for half in range(2):
                fsl = slice(half * 512, half * 512 + 512)
                nc.tensor.matmul(
                    halves[half],
                    lhsT=xgtYU[:, b, nsl],
                    rhs=wfs[:, b % 2, fsl],
                    start=True,
                    stop=False,
                )
                nc.tensor.matmul(
                    halves[half],
                    lhsT=xgtX[p0 : p0 + DIN, b // 2, nsl],
                    rhs=wfs[p0 : p0 + DIN, 2, fsl],
                    start=False,
                    stop=True,
                )

        def unit_pair(nt, bpair, pidx):
            """Two combine units (nt, b0) (nt, b1); zs on ACT per unit, then
            one pair-batched ze and d-reduce tree on DVE or Pool."""
            pn = _tsz(nt)
            path = PAIRS[pidx]
            zs2 = wrk.tile([P, 2, DO], bf16, tag="zs", name="zs2", bufs=3)
            for j, b in enumerate(bpair):
                pZ = ps.tile([P, DO], f32, tag=f"Z{zring[0] % 3}", name="pZ")
                zring[0] += 1
                unit_mms(nt, b, [pZ[:pn, 0:512], pZ[:pn, 512:1024]])
                nc.scalar.activation(zs2[:pn, j, :], pZ[:pn, :], AF.Copy)
            esl = ebs[:pn, nt, 0:EMB]
            eeB2 = bass.AP(
                tensor=esl.tensor,
                offset=esl.offset,
                ap=[esl.ap[0], [0, 2], [0, DOUT], esl.ap[1]],
            )
            ze2 = wrk.tile([P, 2, DOUT, EMB], bf16, tag="ze", name="ze2", bufs=3)
            nc.vector.tensor_tensor(
                ze2[:pn], zs2[:pn].rearrange("p b (o d) -> p b o d", d=EMB),
                eeB2, OP.mult,
            )
            eng = nc.gpsimd if path == "h" else nc.vector
            tg = path
            ob = ob_for(nt)
            t8 = wrk.tile([P, 2, DOUT, 8], bf16, tag=f"t8{tg}", name="t8", bufs=2)
            eng.tensor_tensor(t8[:pn], ze2[:pn, :, :, 0:8], ze2[:pn, :, :, 8:16], OP.add)
            t4 = wrk.tile([P, 2, DOUT, 4], bf16, tag=f"t4{tg}", name="t4", bufs=2)
            eng.tensor_tensor(t4[:pn], t8[:pn, :, :, 0:4], t8[:pn, :, :, 4:8], OP.add)
            t2 = wrk.tile([P, 2, DOUT, 2], bf16, tag=f"t2{tg}", name="t2", bufs=2)
            eng.tensor_tensor(t2[:pn], t4[:pn, :, :, 0:2], t4[:pn, :, :, 2:4], OP.add)
            with nc.allow_low_precision(reason="16-term bf16 reduce"):
                for j, b in enumerate(bpair):
                    eng.tensor_tensor(
                        ob[:pn, b, :].rearrange("p (o v) -> p o v", v=1),
                        t2[:pn, j, :, 0:1],
                        t2[:pn, j, :, 1:2],
                        OP.add,
                    )

        def tail_unit(nt, b, tpath):
            """Unbatched tail unit on the 3-deep Z ring, drains spread."""
            pn = _tsz(nt)
            pZ = ps.tile([P, DO], f32, tag=f"Z{zring[0] % 3}", name="pZt")
            zring[0] += 1
            unit_mms(nt, b, [pZ[:pn, 0:512], pZ[:pn, 512:1024]])
            esl = ebs[:pn, nt, 0:EMB]
            eeB = bass.AP(
                tensor=esl.tensor,
                offset=esl.offset,
                ap=[esl.ap[0], [0, DOUT], esl.ap[1]],
            )
            ze = wrk.tile([P, DOUT, EMB], bf16, tag="ze", name="zet", bufs=3)
            if tpath in ("f", "p"):
                nc.vector.tensor_tensor(
                    ze[:pn], pZ[:pn, :].rearrange("p (o d) -> p o d", d=EMB),
                    eeB, OP.mult,
                )
            else:
                zs = wrk.tile([P, DO], bf16, tag="zs", name="zst", bufs=3)
                nc.scalar.activation(zs[:pn, :], pZ[:pn, :], AF.Copy)
                nc.vector.tensor_tensor(
                    ze[:pn], zs[:pn, :].rearrange("p (o d) -> p o d", d=EMB),
                    eeB, OP.mult,
                )
            eng = nc.gpsimd if tpath in ("b", "p") else nc.vector
            tg = "h" if tpath in ("b", "p") else "g"
            ob = ob_for(nt)
            t8 = wrk.tile([P, DOUT, 8], bf16, tag=f"t8{tg}", name="t8t", bufs=2)
            eng.tensor_tensor(t8[:pn], ze[:pn, :, 0:8], ze[:pn, :, 8:16], OP.add)
            t4 = wrk.tile([P, DOUT, 4], bf16, tag=f"t4{tg}", name="t4t", bufs=2)
            eng.tensor_tensor(t4[:pn], t8[:pn, :, 0:4], t8[:pn, :, 4:8], OP.add)
            t2 = wrk.tile([P, DOUT, 2], bf16, tag=f"t2{tg}", name="t2t", bufs=2)
            eng.tensor_tensor(t2[:pn], t4[:pn, :, 0:2], t4[:pn, :, 2:4], OP.add)
            with nc.allow_low_precision(reason="16-term bf16 reduce"):
                eng.tensor_tensor(
                    ob[:pn, b, :].rearrange("p (o v) -> p o v", v=1),
                    t2[:pn, :, 0:1],
                    t2[:pn, :, 1:2],
                    OP.add,
                )

        # ---- pipeline ----
        pending = []   # (nt, bpair, pidx)
        pcount = [0]

        def enqueue_chunk(c, last_tiles=None):
            for t in range(CNT[c]):
                nt = CT0[c] + t
                if last_tiles is not None and nt not in last_tiles:
                    continue
                for bpair in ((1, 0), (3, 2)):
                    pending.append((nt, bpair, pcount[0]))
                    pcount[0] += 1

        def emit_pair():
            if pending:
                nt, bpair, pidx = pending.pop(0)
                unit_pair(nt, bpair, pidx)
                if bpair[0] == 3:
                    finish_tile(nt)
                return True
            return False

        def hop_chunk(c, slots):
            """Emit chunk c's hop matmuls; `slots` = stripe indices after
            which one pending pair is emitted (or warmup sprinkles early)."""
            pH = ps.tile([P, 4, 256], f32, tag="H", name=f"pH{c}")
            for mt in range(12):
                for acc in range(4):
                    hop_mm(pH, c, acc, mt)
                if mt in slots:
                    if not emit_pair() and c <= 2:
                        warm_mms(SPRINKLE)
            for gi, accs in enumerate(((0, 2), (1, 3))):
                for acc in accs:
                    for mt in range(12, 16):
                        hop_mm(pH, c, acc, mt)
                pair_drains(pH, c, gi)
                if 12 + 2 * gi in slots:
                    if not emit_pair() and c <= 2:
                        warm_mms(SPRINKLE)

        # chunks 0-2: hop-only (stream-paced; sprinkles fill); after chunk 2
        # the misc DMAs have landed -> burst the buffered pairs, then 1:1.
        hop_chunk(0, (1, 3, 5, 7, 9, 11))   # all sprinkle slots: DMA-paced
        enqueue_chunk(0)
        hop_chunk(1, (1, 5, 9))          # sprinkle slots (pairs not ready yet)
        enqueue_chunk(1)
        hop_chunk(2, (1, 5, 9))
        enqueue_chunk(2)
        for _ in range(8):               # burst: chunks 0-1 pairs
            emit_pair()
        for c in range(3, NCH):
            hop_chunk(c, (1, 5, 9, 12))
            enqueue_chunk(c, last_tiles=None if c < NCH - 1 else {CT0[c]})
        while emit_pair():
            pass
        # tail: last tile, 4 unbatched units on the Z ring
        tnt = NT - 1
        for k, b in enumerate((1, 0, 3, 2)):
            tail_unit(tnt, b, TAILP[k])
        finish_tile(tnt)

    nc.compile()
    return nc


_NC_CACHE: list = []


def _get_nc():
    if not _NC_CACHE:
        _NC_CACHE.append(_build())
    return _NC_CACHE[0]


def _prep_shared(node_embeddings, nodevec1, nodevec2, weights_pool, bias_pool):
    nv1 = np.asarray(nodevec1, np.float32)
    nv2 = np.asarray(nodevec2, np.float32)
    z = np.maximum(nv1 @ nv2, 0.0)
    e = np.exp(z - z.max(axis=1, keepdims=True))
    s = e / e.sum(axis=1, keepdims=True)
    s2 = s @ s
    std = np.zeros((NPAD, N), np.float32)
    std[:N] = s.T
    s2d = np.zeros((NPAD, N), np.float32)
    s2d[:N] = s2.T

    wp = np.asarray(weights_pool, np.float32)  # [EMB, K, I, O]

    def blk(M):  # [EMB, I, O] -> [I, (O, EMB)] d-minor
        return np.transpose(M, (1, 2, 0)).reshape(DIN, DO)

    A = blk(wp[:, 0] - wp[:, 2])
    Bb = blk(wp[:, 1])
    C = blk(2.0 * wp[:, 2])
    wf3 = np.stack(
        [np.vstack([Bb, C]), np.vstack([C, Bb]), np.vstack([A, A])], axis=0
    )

    emb = np.asarray(node_embeddings, np.float32)
    ebd = np.zeros((NPAD, EMB + DOUT), np.float32)
    ebd[:N, :EMB] = emb
    ebd[:N, EMB:] = emb @ np.asarray(bias_pool, np.float32)
    return {
        "std": std.astype(BF16),
        "s2d": s2d.astype(BF16),
        "wf3": wf3.astype(BF16),
        "ebd": ebd.astype(BF16),
    }


def _prep_core(x, core):
    xl = np.asarray(x[core * BLOC : (core + 1) * BLOC], np.float32)  # [4, N, 64]
    xw = np.ascontiguousarray(xl.transpose(1, 0, 2).reshape(N, BLOC * DIN))
    xwx = np.zeros((NPAD, 320), np.float32)
    xwx[:N, 0:256] = xw
    xwx[:N, 256:320] = xw[:, 0:64]
    xtp = np.ascontiguousarray(xl.transpose(0, 2, 1).reshape(2, P, N))
    return {"xwx": xwx.astype(BF16), "xtp": xtp.astype(BF16)}


def run(x, node_embeddings, nodevec1, nodevec2, weights_pool, bias_pool, **spmd_kwargs):
    nc = _get_nc()
    shared = _prep_shared(node_embeddings, nodevec1, nodevec2, weights_pool, bias_pool)
    in_maps = [{**shared, **_prep_core(x, c)} for c in range(CORES)]
    res = run_bass_kernel_spmd(nc, in_maps, core_ids=list(range(CORES)), **spmd_kwargs)
    out = np.concatenate(
        [
            np.asarray(res.results[c]["out"], np.float32).transpose(1, 0, 2)
            for c in range(CORES)
        ],
        axis=0,
    )
    return np.ascontiguousarray(out), res


def kernel(x, node_embeddings, nodevec1, nodevec2, weights_pool, bias_pool):
    out, _ = run(x, node_embeddings, nodevec1, nodevec2, weights_pool, bias_pool)
    return out
